# revision 10
# baseline (speedup 1.0000x reference)
"""Distributed Trainium2 kernel for nn_Attention_21990232555717.

Reference (per batch element a, seq b=1024, model dim c=1024, 16 heads):
    qkv = x @ w_qkv                       # (b, 3072)
    q,k,v split per head (hd=64)
    scores = q @ k.T * (1/sqrt(1024))     # (h, b, b)
    attn = softmax(scores, axis=HEADS)    # normalize across the 16 heads!
    out = attn @ v -> (b, 1024) @ w_out + b_out

Sharding: pure data parallel - batch (8) across 8 cores, weights replicated.
No collectives needed.

Per-core dataflow (f32r matmuls for projections, bf16 for the softmax path,
f32 accumulation in PSUM everywhere):
  xT   (c, s) f32r  from PE transposes of x
  QKT  (f, s) bf16  = w_qk^T @ x^T  (lhsT=w_qk f32r, rhs=xT f32r)
  Vb   (s, f) bf16  = x @ w_v       (lhsT=xT, rhs=w_v)
  scoresT (k, q) psum f32 per head  (lhsT=KT_h bf16, rhs=QT_h bf16)
  E = exp(scores/32) bf16; denom = sum_h E; attn = E * recip(denom)  [in-place]
  outT (f=h*64+d, q) f32r = accum_k (lhsT=Vb_h bf16, rhs=attn_h bf16)
  y (s, e) = (lhsT=outT f32r, rhs=w_out f32r) + ones^T b_out
"""

import numpy as np

import concourse.bass as bass
import concourse.mybir as mybir
import concourse.tile as tile
from concourse import bacc
from concourse.bass_utils import run_bass_kernel_spmd
from concourse.masks import make_identity

F32 = mybir.dt.float32
F32R = mybir.dt.float32r
BF16 = mybir.dt.bfloat16
Exp = mybir.ActivationFunctionType.Exp

S = 1024      # sequence length per core (batch element)
C = 1024      # model dim
H = 16        # heads
HD = 64       # head dim
SCALE = 1.0 / (C ** 0.5)
QB = 256      # q block size
NQB = S // QB          # 4 q blocks
NKT = S // 128         # 8 k tiles
NCT = C // 128         # 8 contraction tiles
NG = 4                 # k-tile groups of 2 per q block


def build():
    nc = bacc.Bacc(None, target_bir_lowering=False)
    x_ext = nc.declare_dram_parameter("x", [S, C], F32, isOutput=False)
    wqkv_ext = nc.declare_dram_parameter("w_qkv", [C, 3 * C], F32, isOutput=False)
    wout_ext = nc.declare_dram_parameter("w_out", [C, C], F32, isOutput=False)
    b_ext = nc.declare_dram_parameter("b_out", [C], F32, isOutput=False)
    out_ext = nc.declare_dram_parameter("out", [S, C], F32, isOutput=True)

    wqkv_r = wqkv_ext[:].bitcast(F32R)
    wout_r = wout_ext[:].bitcast(F32R)

    with tile.TileContext(nc) as tc:
        with (
            tc.tile_pool(name="const_p", bufs=1) as const_p,
            tc.tile_pool(name="act_p", bufs=1) as act_p,
        ):
            # ---- constants ----
            ident = const_p.tile([128, 128], F32)
            make_identity(nc, ident)
            ones1_f = const_p.tile([1, 128], F32)
            nc.vector.memset(ones1_f, 1.0)
            ones1 = const_p.tile([1, 128], F32R)
            nc.vector.tensor_copy(ones1, ones1_f)
            b_sb = const_p.tile([1, C], F32R)
            nc.sync.dma_start(b_sb, b_ext[None, :].bitcast(F32R))

            # ---- persistent activations ----
            QKT = act_p.tile([128, H, S], BF16)        # 4 MB  (Q tiles 0..7, K tiles 8..15)
            Vb = act_p.tile([128, NKT, C], BF16)       # 2 MB

            # ============ stages A-C: transpose x, qkv projections ============
            with (
                tc.tile_pool(name="ps_t", bufs=2, space="PSUM") as ps_t,
                tc.tile_pool(name="ps_b", bufs=2, space="PSUM") as ps_b,
                tc.tile_pool(name="xt_p", bufs=1) as xt_p,
                tc.tile_pool(name="xs_p", bufs=2) as xs_p,
                tc.tile_pool(name="w_p", bufs=1) as w_p,
            ):
                xT = xt_p.tile([128, NCT, S], F32R)        # 4 MB
                wqk = w_p.tile([128, NCT, 2 * C], F32R)    # 8 MB
                wv = w_p.tile([128, NCT, C], F32R)         # 4 MB

                with nc.named_scope("load_weights"):
                    for ct in range(NCT):
                        nc.sync.dma_start(
                            wqk[:, ct, :], wqkv_r[ct * 128:(ct + 1) * 128, 0:2 * C])
                        nc.sync.dma_start(
                            wv[:, ct, :], wqkv_r[ct * 128:(ct + 1) * 128, 2 * C:3 * C])

                with nc.named_scope("transpose_x"):
                    for st in range(NKT):
                        xs = xs_p.tile([128, C], F32, tag="xslab")
                        nc.sync.dma_start(xs, x_ext[st * 128:(st + 1) * 128, :])
                        for ct in range(NCT):
                            pt = ps_t.tile([128, 128], F32)
                            nc.tensor.transpose(pt, xs[:, ct * 128:(ct + 1) * 128], ident)
                            if ct % 2 == 0:
                                nc.vector.tensor_copy(xT[:, ct, st * 128:(st + 1) * 128], pt)
                            else:
                                nc.scalar.copy(xT[:, ct, st * 128:(st + 1) * 128], pt)

                # ---- stage B: QKT = w_qk^T @ x^T ----
                with nc.named_scope("qk_proj"):
                    for ft in range(H):
                        pss = [ps_b.tile([128, 512], F32, tag=f"psb{sb}", name=f"psb{ft}_{sb}")
                               for sb in range(2)]
                        for ct in range(NCT):
                            lhsT = wqk[:, ct, ft * 128:(ft + 1) * 128]
                            for sb in range(2):
                                nc.tensor.matmul(
                                    pss[sb], lhsT, xT[:, ct, sb * 512:(sb + 1) * 512],
                                    start=(ct == 0), stop=(ct == NCT - 1),
                                )
                        for sb in range(2):
                            if ft % 2 == 0:
                                nc.scalar.copy(QKT[:, ft, sb * 512:(sb + 1) * 512], pss[sb])
                            else:
                                nc.vector.tensor_copy(QKT[:, ft, sb * 512:(sb + 1) * 512], pss[sb])

                # ---- stage C: Vb = x @ w_v ----
                with nc.named_scope("v_proj"):
                    for st in range(NKT):
                        pss = [ps_b.tile([128, 512], F32, tag=f"psb{fb}", name=f"psc{st}_{fb}")
                               for fb in range(2)]
                        for ct in range(NCT):
                            lhsT = xT[:, ct, st * 128:(st + 1) * 128]
                            for fb in range(2):
                                nc.tensor.matmul(
                                    pss[fb], lhsT, wv[:, ct, fb * 512:(fb + 1) * 512],
                                    start=(ct == 0), stop=(ct == NCT - 1),
                                )
                        for fb in range(2):
                            if st % 2 == 0:
                                nc.scalar.copy(Vb[:, st, fb * 512:(fb + 1) * 512], pss[fb])
                            else:
                                nc.vector.tensor_copy(Vb[:, st, fb * 512:(fb + 1) * 512], pss[fb])

            # ================= stage D/E: attention + out proj =================
            with (
                tc.tile_pool(name="ps_s", bufs=3, space="PSUM") as ps_s,
                tc.tile_pool(name="ps_o", bufs=1, space="PSUM") as ps_o,
                tc.tile_pool(name="ps_y", bufs=1, space="PSUM") as ps_y,
                tc.tile_pool(name="wout_p", bufs=1) as wout_p,
                tc.tile_pool(name="e_pool", bufs=1) as e_pool,
                tc.tile_pool(name="d_pool", bufs=2) as d_pool,
                tc.tile_pool(name="o_pool", bufs=2) as o_pool,
                tc.tile_pool(name="y_pool", bufs=3) as y_pool,
            ):
                wout = wout_p.tile([128, NCT, C], F32R)    # 4 MB
                with nc.named_scope("load_wout"):
                    for ft in range(NCT):
                        nc.sync.dma_start(wout[:, ft, :], wout_r[ft * 128:(ft + 1) * 128, :])

                for qb in range(NQB):
                    q0 = qb * QB
                    Etiles = {}
                    with nc.named_scope(f"attn_qb{qb}"):
                        for g in range(NG):
                            # ---- D1: scores + exp ----
                            pss = ps_s.tile([128, 2, QB], F32, tag="scores")
                            etile_list = []
                            for h in range(H):
                                po = 64 * (h % 2)
                                for j in range(2):
                                    kt = 2 * g + j
                                    lhsT = QKT[po:po + 64, 8 + h // 2, kt * 128:(kt + 1) * 128]
                                    rhs = QKT[po:po + 64, h // 2, q0:q0 + QB]
                                    nc.tensor.matmul(pss[:, j, :], lhsT, rhs,
                                                     start=True, stop=True)
                                et = e_pool.tile([128, 2, QB], BF16, tag=f"E{h}_{g}",
                                                 name=f"E{h}_{g}")
                                nc.scalar.activation(et, pss, Exp, scale=SCALE)
                                Etiles[(h, g)] = et
                                etile_list.append(et)
                                if h < H - 1:
                                    pss = ps_s.tile([128, 2, QB], F32, tag="scores")
                            # ---- D2: denominator + normalize ----
                            lvl = etile_list
                            di = 0
                            while len(lvl) > 1:
                                nxt = []
                                for i in range(0, len(lvl), 2):
                                    if len(lvl) == 2:
                                        dd = d_pool.tile([128, 2, QB], F32, tag="denf",
                                                         name="denf")
                                    else:
                                        dd = d_pool.tile([128, 2, QB], BF16, tag="dent",
                                                         name=f"dent{di}", bufs=10)
                                    eng = nc.vector if di % 3 != 2 else nc.gpsimd
                                    eng.tensor_add(dd, lvl[i], lvl[i + 1])
                                    nxt.append(dd)
                                    di += 1
                                lvl = nxt
                            rec_f = d_pool.tile([128, 2, QB], F32, tag="recf")
                            nc.vector.reciprocal(rec_f, lvl[0])
                            rec = d_pool.tile([128, 2, QB], BF16, tag="rec")
                            nc.vector.tensor_copy(rec, rec_f)
                            for h in range(H):
                                et = Etiles[(h, g)]
                                eng = nc.vector if h % 2 == 0 else nc.gpsimd
                                eng.tensor_mul(et, et, rec)
                        # ---- D3: attn @ v in 4 waves of 4 heads ----
                        # one psum bank per head per wave: a single accumulation
                        # group per 2KB zero region (start=True zeroes the whole
                        # region, so interleaved per-head groups in one bank
                        # would corrupt each other)
                        outT = o_pool.tile([128, NCT, QB], F32R, tag="outT")
                        for w in range(4):
                            aw = ps_o.tile([128, 4, 512], F32, tag="acc",
                                           name=f"acc{qb}_{w}")
                            for kt in range(NKT):
                                g, j = kt // 2, kt % 2
                                for i in range(4):
                                    h = 4 * w + i
                                    po = 64 * (h % 2)
                                    nc.tensor.matmul(
                                        aw[po:po + 64, i, 0:QB],
                                        Vb[:, kt, h * HD:(h + 1) * HD],
                                        Etiles[(h, g)][:, j, :],
                                        start=(kt == 0), stop=(kt == NKT - 1),
                                        tile_position=(0, po),
                                    )
                            for i in range(4):
                                h = 4 * w + i
                                po = 64 * (h % 2)
                                if i % 2 == 0:
                                    nc.vector.tensor_copy(
                                        outT[po:po + 64, h // 2, :], aw[po:po + 64, i, 0:QB])
                                else:
                                    nc.scalar.copy(
                                        outT[po:po + 64, h // 2, :], aw[po:po + 64, i, 0:QB])
                    # ---- out projection ----
                    with nc.named_scope(f"out_proj_qb{qb}"):
                        for qsub in range(QB // 128):
                            for ec in range(2):
                                psy = ps_y.tile([128, 512], F32, tag="psy")
                                for ft in range(NCT):
                                    nc.tensor.matmul(
                                        psy,
                                        outT[:, ft, qsub * 128:(qsub + 1) * 128],
                                        wout[:, ft, ec * 512:(ec + 1) * 512],
                                        start=(ft == 0), stop=False,
                                    )
                                nc.tensor.matmul(
                                    psy, ones1, b_sb[:, ec * 512:(ec + 1) * 512],
                                    start=False, stop=True,
                                )
                                y = y_pool.tile([128, 512], F32, tag="y")
                                nc.scalar.copy(y, psy)
                                nc.sync.dma_start(
                                    out_ext[q0 + qsub * 128:q0 + (qsub + 1) * 128,
                                            ec * 512:(ec + 1) * 512],
                                    y,
                                )

    nc.compile()
    return nc


_NC = None


def _get_nc():
    global _NC
    if _NC is None:
        _NC = build()
    return _NC


def kernel(x, w_qkv, w_out, b_out):
    nc = _get_nc()
    x = np.ascontiguousarray(np.asarray(x, dtype=np.float32))
    w_qkv = np.ascontiguousarray(np.asarray(w_qkv, dtype=np.float32))
    w_out = np.ascontiguousarray(np.asarray(w_out, dtype=np.float32))
    b_out = np.ascontiguousarray(np.asarray(b_out, dtype=np.float32))
    in_maps = [
        {"x": x[i], "w_qkv": w_qkv, "w_out": w_out, "b_out": b_out}
        for i in range(8)
    ]
    res = run_bass_kernel_spmd(nc, in_maps, core_ids=list(range(8)))
    out = np.stack([np.asarray(res.results[i]["out"]) for i in range(8)])
    return out.astype(np.float32)


# revision 12
# speedup vs baseline: 1.0803x; 1.0803x over previous
"""Distributed Trainium2 kernel for nn_Attention_21990232555717.

Reference (per batch element a, seq b=1024, model dim c=1024, 16 heads):
    qkv = x @ w_qkv                       # (b, 3072)
    q,k,v split per head (hd=64)
    scores = q @ k.T * (1/sqrt(1024))     # (h, b, b)
    attn = softmax(scores, axis=HEADS)    # normalize across the 16 heads!
    out = attn @ v -> (b, 1024) @ w_out + b_out

Sharding: pure data parallel - batch (8) across 8 cores, weights replicated.
No collectives needed.

Per-core dataflow (f32r matmuls for projections, bf16 for the softmax path,
f32 accumulation in PSUM everywhere):
  xT   (c, s) f32r  from PE transposes of x
  QKT  (f, s) bf16  = w_qk^T @ x^T  (lhsT=w_qk f32r, rhs=xT f32r)
  Vb   (s, f) bf16  = x @ w_v       (lhsT=xT, rhs=w_v)
  scoresT (k, q) psum f32 per head  (lhsT=KT_h bf16, rhs=QT_h bf16)
  E = exp(scores/32) bf16; denom = sum_h E; attn = E * recip(denom)  [in-place]
  outT (f=h*64+d, q) f32r = accum_k (lhsT=Vb_h bf16, rhs=attn_h bf16)
  y (s, e) = (lhsT=outT f32r, rhs=w_out f32r) + ones^T b_out
"""

import numpy as np

import concourse.bass as bass
import concourse.mybir as mybir
import concourse.tile as tile
from concourse import bacc
from concourse.bass_utils import run_bass_kernel_spmd
from concourse.masks import make_identity

F32 = mybir.dt.float32
F32R = mybir.dt.float32r
BF16 = mybir.dt.bfloat16
Exp = mybir.ActivationFunctionType.Exp

S = 1024      # sequence length per core (batch element)
C = 1024      # model dim
H = 16        # heads
HD = 64       # head dim
SCALE = 1.0 / (C ** 0.5)
QB = 256      # q block size
NQB = S // QB          # 4 q blocks
NKT = S // 128         # 8 k tiles
NCT = C // 128         # 8 contraction tiles
NG = 4                 # k-tile groups of 2 per q block


def build():
    nc = bacc.Bacc(None, target_bir_lowering=False)
    x_ext = nc.declare_dram_parameter("x", [S, C], F32, isOutput=False)
    wqkv_ext = nc.declare_dram_parameter("w_qkv", [C, 3 * C], F32, isOutput=False)
    wout_ext = nc.declare_dram_parameter("w_out", [C, C], F32, isOutput=False)
    b_ext = nc.declare_dram_parameter("b_out", [C], F32, isOutput=False)
    out_ext = nc.declare_dram_parameter("out", [S, C], F32, isOutput=True)

    wqkv_r = wqkv_ext[:].bitcast(F32R)
    wout_r = wout_ext[:].bitcast(F32R)

    with tile.TileContext(nc) as tc:
        with (
            tc.tile_pool(name="const_p", bufs=1) as const_p,
            tc.tile_pool(name="act_p", bufs=1) as act_p,
        ):
            # ---- constants ----
            ident = const_p.tile([128, 128], F32)
            make_identity(nc, ident)
            ones1 = const_p.tile([1, 128], BF16)
            nc.vector.memset(ones1, 1.0)
            b_f = const_p.tile([1, C], F32)
            nc.sync.dma_start(b_f, b_ext[None, :])
            b_sb = const_p.tile([1, C], BF16)
            nc.vector.tensor_copy(b_sb, b_f)

            # ---- persistent activations ----
            QKT = act_p.tile([128, H, S], BF16)        # 4 MB  (Q tiles 0..7, K tiles 8..15)
            Vb = act_p.tile([128, NKT, C], BF16)       # 2 MB

            # ============ stages A-C: transpose x, qkv projections ============
            with (
                tc.tile_pool(name="ps_t", bufs=2, space="PSUM") as ps_t,
                tc.tile_pool(name="ps_b", bufs=2, space="PSUM") as ps_b,
                tc.tile_pool(name="xt_p", bufs=1) as xt_p,
                tc.tile_pool(name="xs_p", bufs=2) as xs_p,
                tc.tile_pool(name="w_p", bufs=1) as w_p,
            ):
                xT = xt_p.tile([128, NCT, S], F32R)        # 4 MB
                wqk = w_p.tile([128, NCT, 2 * C], F32R)    # 8 MB
                wv = w_p.tile([128, NCT, C], F32R)         # 4 MB

                with nc.named_scope("load_weights"):
                    for ct in range(NCT):
                        nc.sync.dma_start(
                            wqk[:, ct, :], wqkv_r[ct * 128:(ct + 1) * 128, 0:2 * C])
                        nc.sync.dma_start(
                            wv[:, ct, :], wqkv_r[ct * 128:(ct + 1) * 128, 2 * C:3 * C])

                with nc.named_scope("transpose_x"):
                    for st in range(NKT):
                        xs = xs_p.tile([128, C], F32, tag="xslab")
                        nc.sync.dma_start(xs, x_ext[st * 128:(st + 1) * 128, :])
                        for ct in range(NCT):
                            pt = ps_t.tile([128, 128], F32)
                            nc.tensor.transpose(pt, xs[:, ct * 128:(ct + 1) * 128], ident)
                            if ct % 2 == 0:
                                nc.vector.tensor_copy(xT[:, ct, st * 128:(st + 1) * 128], pt)
                            else:
                                nc.scalar.copy(xT[:, ct, st * 128:(st + 1) * 128], pt)

                # ---- stage B: QKT = w_qk^T @ x^T ----
                with nc.named_scope("qk_proj"):
                    for ft in range(H):
                        pss = [ps_b.tile([128, 512], F32, tag=f"psb{sb}", name=f"psb{ft}_{sb}")
                               for sb in range(2)]
                        for ct in range(NCT):
                            lhsT = wqk[:, ct, ft * 128:(ft + 1) * 128]
                            for sb in range(2):
                                nc.tensor.matmul(
                                    pss[sb], lhsT, xT[:, ct, sb * 512:(sb + 1) * 512],
                                    start=(ct == 0), stop=(ct == NCT - 1),
                                )
                        for sb in range(2):
                            if ft % 2 == 0:
                                nc.scalar.copy(QKT[:, ft, sb * 512:(sb + 1) * 512], pss[sb])
                            else:
                                nc.vector.tensor_copy(QKT[:, ft, sb * 512:(sb + 1) * 512], pss[sb])

                # ---- stage C: Vb = x @ w_v ----
                with nc.named_scope("v_proj"):
                    for st in range(NKT):
                        pss = [ps_b.tile([128, 512], F32, tag=f"psb{fb}", name=f"psc{st}_{fb}")
                               for fb in range(2)]
                        for ct in range(NCT):
                            lhsT = xT[:, ct, st * 128:(st + 1) * 128]
                            for fb in range(2):
                                nc.tensor.matmul(
                                    pss[fb], lhsT, wv[:, ct, fb * 512:(fb + 1) * 512],
                                    start=(ct == 0), stop=(ct == NCT - 1),
                                )
                        for fb in range(2):
                            if st % 2 == 0:
                                nc.scalar.copy(Vb[:, st, fb * 512:(fb + 1) * 512], pss[fb])
                            else:
                                nc.vector.tensor_copy(Vb[:, st, fb * 512:(fb + 1) * 512], pss[fb])

            # ================= stage D/E: attention + out proj =================
            with (
                tc.tile_pool(name="ps_s", bufs=2, space="PSUM") as ps_s,
                tc.tile_pool(name="ps_o", bufs=1, space="PSUM") as ps_o,
                tc.tile_pool(name="ps_y", bufs=2, space="PSUM") as ps_y,
                tc.tile_pool(name="wout_p", bufs=1) as wout_p,
                tc.tile_pool(name="e_pool", bufs=1) as e_pool,
                tc.tile_pool(name="d_pool", bufs=1) as d_pool,
                tc.tile_pool(name="o_pool", bufs=2) as o_pool,
                tc.tile_pool(name="y_pool", bufs=2) as y_pool,
            ):
                wout = wout_p.tile([128, NCT, C], BF16)    # 2 MB
                with nc.named_scope("load_wout"):
                    for ft in range(NCT):
                        wt = y_pool.tile([128, C], F32, tag="wtmp", name=f"wt{ft}", bufs=2)
                        nc.sync.dma_start(wt, wout_ext[ft * 128:(ft + 1) * 128, :])
                        nc.gpsimd.tensor_copy(wout[:, ft, :], wt)

                def emit_out_proj(q0, outT):
                    with nc.named_scope(f"out_proj_q{q0}"):
                        for qsub in range(QB // 128):
                            for ec in range(2):
                                psy = ps_y.tile([128, 512], F32, tag="psy",
                                                name=f"psy{q0}_{qsub}_{ec}")
                                for ft in range(NCT):
                                    nc.tensor.matmul(
                                        psy,
                                        outT[:, ft, qsub * 128:(qsub + 1) * 128],
                                        wout[:, ft, ec * 512:(ec + 1) * 512],
                                        start=(ft == 0), stop=False,
                                    )
                                nc.tensor.matmul(
                                    psy, ones1, b_sb[:, ec * 512:(ec + 1) * 512],
                                    start=False, stop=True,
                                )
                                y = y_pool.tile([128, 512], F32, tag="y",
                                                name=f"y{q0}_{qsub}_{ec}")
                                nc.scalar.copy(y, psy)
                                nc.sync.dma_start(
                                    out_ext[q0 + qsub * 128:q0 + (qsub + 1) * 128,
                                            ec * 512:(ec + 1) * 512],
                                    y,
                                )

                pending = None  # deferred out-proj: (q0, outT)
                for qb in range(NQB):
                    q0 = qb * QB
                    Etiles = {}
                    with nc.named_scope(f"attn_qb{qb}"):
                        for gg in range(2):  # groups of 4 k-tiles
                            # ---- D1: scores + exp (one wide ACT op per head) ----
                            pss = ps_s.tile([128, 4, QB], F32, tag="scores",
                                            name=f"sc{qb}_{gg}_0")
                            for h in range(H):
                                po = 64 * (h % 2)
                                rhs = QKT[po:po + 64, h // 2, q0:q0 + QB]
                                for j in range(4):
                                    kt = 4 * gg + j
                                    lhsT = QKT[po:po + 64, 8 + h // 2, kt * 128:(kt + 1) * 128]
                                    nc.tensor.matmul(pss[:, j, :], lhsT, rhs,
                                                     start=True, stop=True)
                                et = e_pool.tile([128, 4, QB], BF16, tag=f"E{h}_{gg}",
                                                 name=f"E{h}_{gg}")
                                nc.scalar.activation(et, pss, Exp, scale=SCALE)
                                Etiles[(h, gg)] = et
                                if h < H - 1:
                                    pss = ps_s.tile([128, 4, QB], F32, tag="scores",
                                                    name=f"sc{qb}_{gg}_{h + 1}")
                            if pending is not None and gg == 0:
                                # slot previous block's out-proj here so the PE has
                                # dense work while D2 runs on DVE/GPSIMD
                                emit_out_proj(*pending)
                                pending = None
                            # ---- D2: denominator + normalize ----
                            lvl = [Etiles[(h, gg)] for h in range(H)]
                            di = 0
                            level = 1
                            lvl_bufs = {1: 8, 2: 4, 3: 2}
                            while len(lvl) > 1:
                                nxt = []
                                for i in range(0, len(lvl), 2):
                                    if len(lvl) == 2:
                                        dd = d_pool.tile([128, 4, QB], F32, tag="denf",
                                                         name="denf", bufs=1)
                                    else:
                                        dd = d_pool.tile([128, 4, QB], BF16, tag=f"dl{level}",
                                                         name=f"dl{level}_{i}",
                                                         bufs=lvl_bufs[level])
                                    eng = nc.vector if di % 3 != 2 else nc.gpsimd
                                    eng.tensor_add(dd, lvl[i], lvl[i + 1])
                                    nxt.append(dd)
                                    di += 1
                                lvl = nxt
                                level += 1
                            rec_f = d_pool.tile([128, 4, QB], F32, tag="recf", bufs=1)
                            nc.vector.reciprocal_approx_fast(out=rec_f, in_=lvl[0])
                            rec = d_pool.tile([128, 4, QB], BF16, tag="rec", bufs=2)
                            nc.vector.tensor_copy(rec, rec_f)
                            for h in range(H):
                                et = Etiles[(h, gg)]
                                eng = nc.vector if h % 3 != 2 else nc.gpsimd
                                eng.tensor_mul(et, et, rec)
                        # ---- D3: attn @ v in 8 waves of 2 heads ----
                        # one psum bank per head per wave: a single accumulation
                        # group per 2KB zero region (start=True zeroes the whole
                        # region, so interleaved per-head groups in one bank
                        # would corrupt each other)
                        outT = o_pool.tile([128, NCT, QB], BF16, tag="outT",
                                           name=f"outT{qb}")
                        for w in range(NKT):
                            aw = ps_o.tile([128, 2, 512], F32, tag="acc",
                                           name=f"acc{qb}_{w}")
                            for kt in range(NKT):
                                gg, j = kt // 4, kt % 4
                                for i in range(2):
                                    h = 2 * w + i
                                    po = 64 * (h % 2)
                                    nc.tensor.matmul(
                                        aw[po:po + 64, i, 0:QB],
                                        Vb[:, kt, h * HD:(h + 1) * HD],
                                        Etiles[(h, gg)][:, j, :],
                                        start=(kt == 0), stop=(kt == NKT - 1),
                                        tile_position=(0, po),
                                    )
                            for i in range(2):
                                h = 2 * w + i
                                po = 64 * (h % 2)
                                if i % 2 == 0:
                                    nc.vector.tensor_copy(
                                        outT[po:po + 64, h // 2, :], aw[po:po + 64, i, 0:QB])
                                else:
                                    nc.scalar.copy(
                                        outT[po:po + 64, h // 2, :], aw[po:po + 64, i, 0:QB])
                    pending = (q0, outT)
                emit_out_proj(*pending)

    nc.compile()
    return nc


_NC = None


def _get_nc():
    global _NC
    if _NC is None:
        _NC = build()
    return _NC


def kernel(x, w_qkv, w_out, b_out):
    nc = _get_nc()
    x = np.ascontiguousarray(np.asarray(x, dtype=np.float32))
    w_qkv = np.ascontiguousarray(np.asarray(w_qkv, dtype=np.float32))
    w_out = np.ascontiguousarray(np.asarray(w_out, dtype=np.float32))
    b_out = np.ascontiguousarray(np.asarray(b_out, dtype=np.float32))
    in_maps = [
        {"x": x[i], "w_qkv": w_qkv, "w_out": w_out, "b_out": b_out}
        for i in range(8)
    ]
    res = run_bass_kernel_spmd(nc, in_maps, core_ids=list(range(8)))
    out = np.stack([np.asarray(res.results[i]["out"]) for i in range(8)])
    return out.astype(np.float32)


# revision 13
# speedup vs baseline: 1.3412x; 1.2415x over previous
"""Distributed Trainium2 kernel for nn_Attention_21990232555717.

Reference (per batch element a, seq b=1024, model dim c=1024, 16 heads):
    qkv = x @ w_qkv                       # (b, 3072)
    q,k,v split per head (hd=64)
    scores = q @ k.T * (1/sqrt(1024))     # (h, b, b)
    attn = softmax(scores, axis=HEADS)    # normalize across the 16 heads!
    out = attn @ v -> (b, 1024) @ w_out + b_out

Sharding: pure data parallel - batch (8) across 8 cores, weights replicated.
No collectives needed.

Per-core dataflow (f32r matmuls for projections, bf16 for the softmax path,
f32 accumulation in PSUM everywhere):
  xT   (c, s) f32r  from PE transposes of x
  QKT  (f, s) bf16  = w_qk^T @ x^T  (lhsT=w_qk f32r, rhs=xT f32r)
  Vb   (s, f) bf16  = x @ w_v       (lhsT=xT, rhs=w_v)
  scoresT (k, q) psum f32 per head  (lhsT=KT_h bf16, rhs=QT_h bf16)
  E = exp(scores/32) bf16; denom = sum_h E; attn = E * recip(denom)  [in-place]
  outT (f=h*64+d, q) f32r = accum_k (lhsT=Vb_h bf16, rhs=attn_h bf16)
  y (s, e) = (lhsT=outT f32r, rhs=w_out f32r) + ones^T b_out
"""

import numpy as np

import concourse.bass as bass
import concourse.mybir as mybir
import concourse.tile as tile
from concourse import bacc
from concourse.bass_utils import run_bass_kernel_spmd
from concourse.masks import make_identity

F32 = mybir.dt.float32
F32R = mybir.dt.float32r
BF16 = mybir.dt.bfloat16
Exp = mybir.ActivationFunctionType.Exp

S = 1024      # sequence length per core (batch element)
C = 1024      # model dim
H = 16        # heads
HD = 64       # head dim
SCALE = 1.0 / (C ** 0.5)
QB = 256      # q block size
NQB = S // QB          # 4 q blocks
NKT = S // 128         # 8 k tiles
NCT = C // 128         # 8 contraction tiles
NG = 4                 # k-tile groups of 2 per q block


def build():
    nc = bacc.Bacc(None, target_bir_lowering=False)
    x_ext = nc.declare_dram_parameter("x", [S, C], F32, isOutput=False)
    wqkv_ext = nc.declare_dram_parameter("w_qkv", [C, 3 * C], F32, isOutput=False)
    wout_ext = nc.declare_dram_parameter("w_out", [C, C], F32, isOutput=False)
    b_ext = nc.declare_dram_parameter("b_out", [C], F32, isOutput=False)
    out_ext = nc.declare_dram_parameter("out", [S, C], F32, isOutput=True)

    wqkv_r = wqkv_ext[:].bitcast(F32R)
    wout_r = wout_ext[:].bitcast(F32R)

    with tile.TileContext(nc) as tc:
        with (
            tc.tile_pool(name="const_p", bufs=1) as const_p,
            tc.tile_pool(name="act_p", bufs=1) as act_p,
        ):
            # ---- constants ----
            ident = const_p.tile([128, 128], F32)
            make_identity(nc, ident)
            ones1 = const_p.tile([1, 128], BF16)
            nc.vector.memset(ones1, 1.0)
            b_f = const_p.tile([1, C], F32)
            nc.sync.dma_start(b_f, b_ext[None, :])
            b_sb = const_p.tile([1, C], BF16)
            nc.vector.tensor_copy(b_sb, b_f)

            # ---- persistent activations ----
            QKT = act_p.tile([128, H, S], BF16)        # 4 MB  (Q tiles 0..7, K tiles 8..15)
            Vb = act_p.tile([128, NKT, C], BF16)       # 2 MB

            # ============ stages A-C: transpose x, qkv projections ============
            with (
                tc.tile_pool(name="ps_t", bufs=2, space="PSUM") as ps_t,
                tc.tile_pool(name="ps_b", bufs=2, space="PSUM") as ps_b,
                tc.tile_pool(name="xt_p", bufs=1) as xt_p,
                tc.tile_pool(name="xs_p", bufs=2) as xs_p,
                tc.tile_pool(name="w_p", bufs=1) as w_p,
            ):
                xT = xt_p.tile([128, NCT, S], F32R)        # 4 MB
                wqk = w_p.tile([128, NCT, 2 * C], F32R)    # 8 MB
                wv = w_p.tile([128, NCT, C], F32R)         # 4 MB

                with nc.named_scope("load_weights"):
                    for ct in range(NCT):
                        nc.sync.dma_start(
                            wqk[:, ct, :], wqkv_r[ct * 128:(ct + 1) * 128, 0:2 * C])
                        nc.sync.dma_start(
                            wv[:, ct, :], wqkv_r[ct * 128:(ct + 1) * 128, 2 * C:3 * C])

                with nc.named_scope("transpose_x"):
                    for st in range(NKT):
                        xs = xs_p.tile([128, C], F32, tag="xslab")
                        nc.sync.dma_start(xs, x_ext[st * 128:(st + 1) * 128, :])
                        for ct in range(NCT):
                            pt = ps_t.tile([128, 128], F32)
                            nc.tensor.transpose(pt, xs[:, ct * 128:(ct + 1) * 128], ident)
                            if ct % 2 == 0:
                                nc.vector.tensor_copy(xT[:, ct, st * 128:(st + 1) * 128], pt)
                            else:
                                nc.scalar.copy(xT[:, ct, st * 128:(st + 1) * 128], pt)

                # ---- stage B: QKT = w_qk^T @ x^T ----
                with nc.named_scope("qk_proj"):
                    for ft in range(H):
                        pss = [ps_b.tile([128, 512], F32, tag=f"psb{sb}", name=f"psb{ft}_{sb}")
                               for sb in range(2)]
                        for ct in range(NCT):
                            lhsT = wqk[:, ct, ft * 128:(ft + 1) * 128]
                            for sb in range(2):
                                nc.tensor.matmul(
                                    pss[sb], lhsT, xT[:, ct, sb * 512:(sb + 1) * 512],
                                    start=(ct == 0), stop=(ct == NCT - 1),
                                )
                        for sb in range(2):
                            if ft % 2 == 0:
                                nc.scalar.copy(QKT[:, ft, sb * 512:(sb + 1) * 512], pss[sb])
                            else:
                                nc.vector.tensor_copy(QKT[:, ft, sb * 512:(sb + 1) * 512], pss[sb])

                # ---- stage C: Vb = x @ w_v ----
                with nc.named_scope("v_proj"):
                    for st in range(NKT):
                        pss = [ps_b.tile([128, 512], F32, tag=f"psb{fb}", name=f"psc{st}_{fb}")
                               for fb in range(2)]
                        for ct in range(NCT):
                            lhsT = xT[:, ct, st * 128:(st + 1) * 128]
                            for fb in range(2):
                                nc.tensor.matmul(
                                    pss[fb], lhsT, wv[:, ct, fb * 512:(fb + 1) * 512],
                                    start=(ct == 0), stop=(ct == NCT - 1),
                                )
                        for fb in range(2):
                            if st % 2 == 0:
                                nc.scalar.copy(Vb[:, st, fb * 512:(fb + 1) * 512], pss[fb])
                            else:
                                nc.vector.tensor_copy(Vb[:, st, fb * 512:(fb + 1) * 512], pss[fb])

            # ================= stage D/E: attention + out proj =================
            with (
                tc.tile_pool(name="ps_s", bufs=2, space="PSUM") as ps_s,
                tc.tile_pool(name="ps_o", bufs=1, space="PSUM") as ps_o,
                tc.tile_pool(name="ps_y", bufs=2, space="PSUM") as ps_y,
                tc.tile_pool(name="wout_p", bufs=1) as wout_p,
                tc.tile_pool(name="e_pool", bufs=1) as e_pool,
                tc.tile_pool(name="d_pool", bufs=1) as d_pool,
                tc.tile_pool(name="o_pool", bufs=2) as o_pool,
                tc.tile_pool(name="y_pool", bufs=2) as y_pool,
            ):
                wout = wout_p.tile([128, NCT, C], BF16)    # 2 MB
                with nc.named_scope("load_wout"):
                    for ft in range(NCT):
                        wt = y_pool.tile([128, C], F32, tag="wtmp", name=f"wt{ft}", bufs=2)
                        nc.sync.dma_start(wt, wout_ext[ft * 128:(ft + 1) * 128, :])
                        nc.vector.tensor_copy(wout[:, ft, :], wt)

                def emit_out_proj(q0, outT):
                    with nc.named_scope(f"out_proj_q{q0}"):
                        for qsub in range(QB // 128):
                            for ec in range(2):
                                psy = ps_y.tile([128, 512], F32, tag="psy",
                                                name=f"psy{q0}_{qsub}_{ec}")
                                for ft in range(NCT):
                                    nc.tensor.matmul(
                                        psy,
                                        outT[:, ft, qsub * 128:(qsub + 1) * 128],
                                        wout[:, ft, ec * 512:(ec + 1) * 512],
                                        start=(ft == 0), stop=False,
                                    )
                                nc.tensor.matmul(
                                    psy, ones1, b_sb[:, ec * 512:(ec + 1) * 512],
                                    start=False, stop=True,
                                )
                                y = y_pool.tile([128, 512], F32, tag="y",
                                                name=f"y{q0}_{qsub}_{ec}")
                                nc.scalar.copy(y, psy)
                                nc.sync.dma_start(
                                    out_ext[q0 + qsub * 128:q0 + (qsub + 1) * 128,
                                            ec * 512:(ec + 1) * 512],
                                    y,
                                )

                pending = None  # deferred out-proj: (q0, outT)
                for qb in range(NQB):
                    q0 = qb * QB
                    Etiles = {}
                    with nc.named_scope(f"attn_qb{qb}"):
                        for gg in range(2):  # groups of 4 k-tiles
                            # ---- D1: scores + exp (one wide ACT op per head) ----
                            pss = ps_s.tile([128, 4 * QB], F32, tag="scores",
                                            name=f"sc{qb}_{gg}_0")
                            for h in range(H):
                                po = 64 * (h % 2)
                                rhs = QKT[po:po + 64, h // 2, q0:q0 + QB]
                                for j in range(4):
                                    kt = 4 * gg + j
                                    lhsT = QKT[po:po + 64, 8 + h // 2, kt * 128:(kt + 1) * 128]
                                    nc.tensor.matmul(pss[:, j * QB:(j + 1) * QB], lhsT, rhs,
                                                     start=True, stop=True)
                                et = e_pool.tile([128, 4 * QB], BF16, tag=f"E{h}_{gg}",
                                                 name=f"E{h}_{gg}")
                                nc.scalar.activation(et, pss, Exp, scale=SCALE)
                                Etiles[(h, gg)] = et
                                if h < H - 1:
                                    pss = ps_s.tile([128, 4 * QB], F32, tag="scores",
                                                    name=f"sc{qb}_{gg}_{h + 1}")
                            if pending is not None and gg == 0:
                                # slot previous block's out-proj here so the PE has
                                # dense work while D2 runs on DVE/GPSIMD
                                emit_out_proj(*pending)
                                pending = None
                            # ---- D2: denominator + normalize ----
                            lvl = [Etiles[(h, gg)] for h in range(H)]
                            di = 0
                            level = 1
                            lvl_bufs = {1: 8, 2: 4, 3: 2}
                            while len(lvl) > 1:
                                nxt = []
                                for i in range(0, len(lvl), 2):
                                    if len(lvl) == 2:
                                        dd = d_pool.tile([128, 4 * QB], F32, tag="denf",
                                                         name="denf", bufs=1)
                                    else:
                                        dd = d_pool.tile([128, 4 * QB], BF16, tag=f"dl{level}",
                                                         name=f"dl{level}_{i}",
                                                         bufs=lvl_bufs[level])
                                    nc.vector.tensor_add(dd, lvl[i], lvl[i + 1])
                                    nxt.append(dd)
                                    di += 1
                                lvl = nxt
                                level += 1
                            rec_f = d_pool.tile([128, 4 * QB], F32, tag="recf", bufs=1)
                            nc.vector.reciprocal_approx_fast(out=rec_f, in_=lvl[0])
                            rec = d_pool.tile([128, 4 * QB], BF16, tag="rec", bufs=2)
                            nc.vector.tensor_copy(rec, rec_f)
                            for h in range(H):
                                et = Etiles[(h, gg)]
                                nc.vector.tensor_mul(et, et, rec)
                        # ---- D3: attn @ v in 8 waves of 2 heads ----
                        # one psum bank per head per wave: a single accumulation
                        # group per 2KB zero region (start=True zeroes the whole
                        # region, so interleaved per-head groups in one bank
                        # would corrupt each other)
                        outT = o_pool.tile([128, NCT, QB], BF16, tag="outT",
                                           name=f"outT{qb}")
                        for w in range(NKT):
                            aw = ps_o.tile([128, 2, 512], F32, tag="acc",
                                           name=f"acc{qb}_{w}")
                            for kt in range(NKT):
                                gg, j = kt // 4, kt % 4
                                for i in range(2):
                                    h = 2 * w + i
                                    po = 64 * (h % 2)
                                    nc.tensor.matmul(
                                        aw[po:po + 64, i, 0:QB],
                                        Vb[:, kt, h * HD:(h + 1) * HD],
                                        Etiles[(h, gg)][:, j * QB:(j + 1) * QB],
                                        start=(kt == 0), stop=(kt == NKT - 1),
                                        tile_position=(0, po),
                                    )
                            for i in range(2):
                                h = 2 * w + i
                                po = 64 * (h % 2)
                                if i % 2 == 0:
                                    nc.vector.tensor_copy(
                                        outT[po:po + 64, h // 2, :], aw[po:po + 64, i, 0:QB])
                                else:
                                    nc.scalar.copy(
                                        outT[po:po + 64, h // 2, :], aw[po:po + 64, i, 0:QB])
                    pending = (q0, outT)
                emit_out_proj(*pending)

    nc.compile()
    return nc


_NC = None


def _get_nc():
    global _NC
    if _NC is None:
        _NC = build()
    return _NC


def kernel(x, w_qkv, w_out, b_out):
    nc = _get_nc()
    x = np.ascontiguousarray(np.asarray(x, dtype=np.float32))
    w_qkv = np.ascontiguousarray(np.asarray(w_qkv, dtype=np.float32))
    w_out = np.ascontiguousarray(np.asarray(w_out, dtype=np.float32))
    b_out = np.ascontiguousarray(np.asarray(b_out, dtype=np.float32))
    in_maps = [
        {"x": x[i], "w_qkv": w_qkv, "w_out": w_out, "b_out": b_out}
        for i in range(8)
    ]
    res = run_bass_kernel_spmd(nc, in_maps, core_ids=list(range(8)))
    out = np.stack([np.asarray(res.results[i]["out"]) for i in range(8)])
    return out.astype(np.float32)
